# revision 15
# baseline (speedup 1.0000x reference)
"""Multi-head causal attention with RoPE on 8 Trainium2 NeuronCores.

Sharding: 2 (batch) x 4 (head-groups of 4 heads). Each core computes
QKV projections, RoPE, flash-style causal attention and its slice of the
output projection for one batch and 4 heads; partial outputs are summed
on the host (row-sharded out_proj => partial-sum reduction).

Device layout choices (everything host-prepped to avoid on-device
transposes, fp32 has no DMA-transpose path):
  - all inputs host-packed so every DMA writes wide contiguous
    per-partition lines (>=4KB): xT in column-group-major chunks,
    W_q/W_k in per-head chunks, wv/wo as single transfers
  - Q^T, K^T computed as [head_dim, S] (lhsT = W tile, rhs = xT)
  - V computed natural [S, head_dim] (lhsT = xT tile, rhs = Wv)
  - scores computed transposed [k, q]; softmax sum over k (partitions)
    via a full-width all-ones stationary matmul, which lands the same sum
    on every PSUM partition so normalization needs no broadcast
  - RoPE rotate-half entirely on DVE via partition-offset muls against a
    host-preshifted signed sin table (no PE matmul)
  - causal diagonal 512-blocks computed at half width (cols [256:512])
    in scores/exp/PV; the half-pair's softmax contribution is folded
    into the DVE pair-sum tree with a partial add
  - attention g-outer; each group's out-proj is emitted after the NEXT
    group's first attention head so the PE queue never stalls on the
    last head's normalization
  - partial outputs row-buffered in SBUF and DMA'd out as bf16 rows
    (host accumulates in fp32)
"""

import math
import sys

import numpy as np

try:
    import concourse.bass as bass  # noqa: F401
except Exception:
    sys.path.insert(0, "/opt/trn_rl_repo")

import ml_dtypes

P = 128
B = 2
S = 2048
D = 2048
H = 16
HEAD = 128
N_CORES = 8
HG = 4            # head groups (tensor-parallel dimension)
HPG = H // HG     # heads per group = 4
DG = HPG * HEAD   # group width = 512
SG = 512          # q-group (free dim) size
DOUT = 2048
DI_T = D // P     # d_in k-tiles = 16
NSG = S // SG     # seq 512-groups = 4

BF16 = ml_dtypes.bfloat16


def _emit(tc, io, cfg, sfx=""):
    """Emit the per-core program. io: dict of dram APs. cfg: sizes."""
    import concourse.mybir as mybir

    nc = tc.nc
    bf = mybir.dt.bfloat16
    f32 = mybir.dt.float32
    Exp = mybir.ActivationFunctionType.Exp

    s = cfg["S"]
    d = cfg["D"]
    dout = cfg["DOUT"]
    di_t = d // P          # d_in k-tiles
    st = s // P            # seq 128-tiles
    nsg = s // SG          # seq 512-groups
    nos = dout // SG       # out column slices
    inv_sqrt_hd = 1.0 / math.sqrt(HEAD)

    # host-packed layouts (see make_in_maps)
    xTg = io["xTg"].rearrange("p (g n) -> p g n", g=nsg)       # n = (o c)
    # wqk: per head, [half][q/k][o-within-half][m] so one DMA covers both
    # weight tensors (and h0 can stream in two halves)
    wqkh = io["wqkh"].rearrange("p (h n) -> p h n", h=HPG)

    const = tc.alloc_tile_pool(name="const" + sfx, bufs=1)
    stores = tc.alloc_tile_pool(name="stores" + sfx, bufs=1)

    cos_sb = const.tile([P, s], bf, tag="cos")
    sinsh_sb = const.tile([P, s], bf, tag="sinsh")
    mask_sb = const.tile([P, HG, SG], bf, tag="mask")
    ones_bf_sb = const.tile([P, P], bf, tag="ones_bf")
    wv_sb = const.tile([P, di_t, DG], bf, tag="wv")
    wo_sb = const.tile([P, HPG, dout], bf, tag="wo")

    # persistent activation stores
    qt_sb = stores.tile([P, HPG, s], bf, tag="qt")
    kt_sb = stores.tile([P, HPG, s], bf, tag="kt")
    v_sb = stores.tile([P, st, DG], bf, tag="v")
    ctx_sb = stores.tile([P, HPG, s], bf, tag="ctx")

    # ---- phase 1: projections + RoPE ----
    with tc.tile_pool(name="xt" + sfx, bufs=1) as xtp, \
         tc.tile_pool(name="wqk" + sfx, bufs=2) as wqkp, \
         tc.tile_pool(name="p1tmp" + sfx, bufs=4) as p1tmp:
        # xt_sb inner layout per column group: (o, c) with c the 512 cols
        xt_sb = xtp.tile([P, nsg, di_t * SG], bf, tag="xt")

        # DMA order = consumption order; each transfer has wide
        # per-partition lines so it runs at full DMA rate.
        hdi = di_t // 2
        hx = hdi * SG
        hw = 2 * hdi * P
        wqk_h0 = wqkp.tile([P, 2, 2, hdi, P], bf, tag="wqk")
        # two HWDGE queues in parallel: weights on sync, x on scalar;
        # h0 weights + first column group stream in halves so the first
        # (split) QK accumulation starts as early as possible
        nc.sync.dma_start(wqk_h0[:, 0], wqkh[:, 0, 0:hw])
        nc.sync.dma_start(xt_sb[:, 0, 0:hx], xTg[:, 0, 0:hx])
        nc.sync.dma_start(wqk_h0[:, 1], wqkh[:, 0, hw:])
        nc.sync.dma_start(xt_sb[:, 0, hx:], xTg[:, 0, hx:])
        nc.sync.dma_start(wv_sb[:], io["wv"][:])
        nc.sync.dma_start(xt_sb[:, 1, :], xTg[:, 1, :])
        nc.sync.dma_start(cos_sb[:], io["cosT"][:])
        nc.sync.dma_start(sinsh_sb[:], io["sinT"][:])
        nc.sync.dma_start(mask_sb[:], io["masks"][:])
        nc.sync.dma_start(ones_bf_sb[:], io["ones_bf"][:])
        for g in range(2, nsg):
            nc.sync.dma_start(xt_sb[:, g, :], xTg[:, g, :])
        nc.sync.dma_start(wo_sb[:], io["wo"][:])

        ps_main = tc.alloc_tile_pool(name="ps_main" + sfx, bufs=3, space="PSUM")
        ps2 = tc.alloc_tile_pool(name="ps2" + sfx, bufs=2, space="PSUM")
        ps_sum = tc.alloc_tile_pool(name="ps_sum" + sfx, bufs=1, space="PSUM")

        def emit_rope(qa, dst, hh, sl):
            # q*cos + rot_half(q)*sin via partition-offset DVE muls;
            # sinsh is host-preshifted: sinsh[64:] = -sin[:64], sinsh[:64] = sin[64:]
            t1 = p1tmp.tile([P, SG], bf, tag="t1")
            nc.vector.tensor_mul(t1, qa, cos_sb[:, sl])
            t2 = p1tmp.tile([P, SG], bf, tag="t2")
            nc.vector.tensor_mul(t2[0:64, :], qa[64:128, :], sinsh_sb[64:128, sl])
            nc.vector.tensor_mul(t2[64:128, :], qa[0:64, :], sinsh_sb[0:64, sl])
            nc.vector.tensor_add(dst[:, hh, sl], t1, t2)

        def emit_qk_group(h, wqk_t, g, split=False):
            sl = slice(g * SG, (g + 1) * SG)
            for wsel, dst in ((0, qt_sb), (1, kt_sb)):
                qa = p1tmp.tile([P, SG], bf, tag="qa")
                if split:
                    # two half accumulations so the first half can start
                    # before the second half's operands arrive
                    pq_a = ps_main.tile([P, SG], f32, tag="ps")
                    for o in range(di_t // 2):
                        nc.tensor.matmul(
                            pq_a,
                            lhsT=wqk_t[:, 0, wsel, o, :],
                            rhs=xt_sb[:, g, o * SG:(o + 1) * SG],
                            start=(o == 0),
                            stop=(o == di_t // 2 - 1),
                        )
                    pq_b = ps_main.tile([P, SG], f32, tag="ps")
                    for o in range(di_t // 2, di_t):
                        nc.tensor.matmul(
                            pq_b,
                            lhsT=wqk_t[:, 1, wsel, o - di_t // 2, :],
                            rhs=xt_sb[:, g, o * SG:(o + 1) * SG],
                            start=(o == di_t // 2),
                            stop=(o == di_t - 1),
                        )
                    qh = p1tmp.tile([P, SG], f32, tag="qh", bufs=2)
                    nc.scalar.copy(qh, pq_a)
                    nc.vector.tensor_add(qa, qh, pq_b)
                else:
                    pq = ps_main.tile([P, SG], f32, tag="ps")
                    for o in range(di_t):
                        nc.tensor.matmul(
                            pq,
                            lhsT=wqk_t[:, o // hdi, wsel, o % hdi, :],
                            rhs=xt_sb[:, g, o * SG:(o + 1) * SG],
                            start=(o == 0),
                            stop=(o == di_t - 1),
                        )
                    nc.scalar.copy(qa, pq)
                emit_rope(qa, dst, h, sl)

        def emit_v(si):
            g, c0 = si // 4, (si % 4) * P
            pv = ps_main.tile([P, SG], f32, tag="ps")
            for o in range(di_t):
                nc.tensor.matmul(
                    pv[:, :DG],
                    lhsT=xt_sb[:, g, o * SG + c0:o * SG + c0 + P],
                    rhs=wv_sb[:, o, :],
                    start=(o == 0),
                    stop=(o == di_t - 1),
                )
            nc.vector.tensor_copy(v_sb[:, si, :], pv[:, :DG])

        # pace PE consumption of xT column groups to DMA arrival: QK h0
        # group g, then the V tiles of the same column group
        for g in range(nsg):
            emit_qk_group(0, wqk_h0, g, split=(g == 0))
            for si in range(4 * g, 4 * (g + 1)):
                emit_v(si)

        for h in range(1, HPG):
            wqk_t = wqkp.tile([P, 2, 2, hdi, P], bf, tag="wqk")
            nc.sync.dma_start(wqk_t[:], wqkh[:, h, :])
            for g in range(nsg):
                emit_qk_group(h, wqk_t, g)

    # ---- phase 2+3: attention with delayed-interleaved output projection --
    # Each group's out-proj is emitted after the next group's first head so
    # the PE queue has ready work while the last head normalizes.
    with tc.tile_pool(name="p2tmp" + sfx, bufs=10) as p2tmp, \
         tc.tile_pool(name="p2rb" + sfx, bufs=3) as p2rb, \
         tc.tile_pool(name="outp" + sfx, bufs=2) as outp:

        def emit_head(g, h):
            qsl = slice(g * SG, (g + 1) * SG)
            jmax = min((g + 1) * SG // P, st)
            pctx = ps_main.tile([P, SG], f32, tag="ps")
            psum_l = ps_sum.tile([P, SG], f32, tag="l")

            # stream 1: paired score MMs + one exp per pair; DVE
            # pre-reduces each full pair so the softmax-sum matmul
            # stream is halved; diagonal half-pair at half width
            ats = []
            acc = None
            for jp in range(0, jmax, 2):
                r = jp - 4 * g
                # per-k-tile causal column offsets (128-granularity on the
                # diagonal 512-block)
                off0 = max(0, r) * P
                off1 = max(0, r + 1) * P
                wsl0 = slice(g * SG + off0, (g + 1) * SG)
                wsl1 = slice(g * SG + off1, (g + 1) * SG)
                ps2t = ps2.tile([P, 2, SG], f32, tag="ps2")
                nc.tensor.matmul(
                    ps2t[:, 0, off0:],
                    lhsT=kt_sb[:, h, jp * P:(jp + 1) * P],
                    rhs=qt_sb[:, h, wsl0],
                    start=True,
                    stop=True,
                )
                nc.tensor.matmul(
                    ps2t[:, 1, off1:],
                    lhsT=kt_sb[:, h, (jp + 1) * P:(jp + 2) * P],
                    rhs=qt_sb[:, h, wsl1],
                    start=True,
                    stop=True,
                )
                at2 = p2tmp.tile([P, 2, SG], bf, tag="at")
                if r < 0:
                    # off-diagonal: one paired exp, no mask
                    nc.scalar.activation(at2[:, :, :], ps2t[:, :, :],
                                         Exp, scale=inv_sqrt_hd)
                    if acc is None:
                        acc = p2tmp.tile([P, SG], bf, tag="dacc")
                        nc.vector.tensor_add(acc, at2[:, 0, :], at2[:, 1, :])
                    else:
                        nc.vector.tensor_add(acc, acc, at2[:, 0, :])
                        nc.vector.tensor_add(acc, acc, at2[:, 1, :])
                else:
                    # diagonal: per-tile width exp + triangle mask
                    nc.scalar.activation(at2[:, 0, off0:], ps2t[:, 0, off0:],
                                         Exp, scale=inv_sqrt_hd)
                    nc.scalar.activation(at2[:, 1, off1:], ps2t[:, 1, off1:],
                                         Exp, scale=inv_sqrt_hd)
                    nc.vector.tensor_mul(at2[:, 0, off0:], at2[:, 0, off0:],
                                         mask_sb[:, r, off0:])
                    nc.vector.tensor_mul(at2[:, 1, off1:], at2[:, 1, off1:],
                                         mask_sb[:, r + 1, off1:])
                    if acc is None:
                        acc = p2tmp.tile([P, SG], bf, tag="dacc")
                        nc.vector.tensor_copy(acc, at2[:, 0, :])
                    else:
                        nc.vector.tensor_add(acc[:, off0:], acc[:, off0:],
                                             at2[:, 0, off0:])
                    nc.vector.tensor_add(acc[:, off1:], acc[:, off1:],
                                         at2[:, 1, off1:])
                ats.append((at2, off0, off1))

            # stream 2: PV accumulation (wait-free after exps drain)
            for idx, (at2, off0, off1) in enumerate(ats):
                for jj, off in ((0, off0), (1, off1)):
                    j = 2 * idx + jj
                    nc.tensor.matmul(
                        pctx[:, off:],
                        lhsT=v_sb[:, j, h * P:(h + 1) * P],
                        rhs=at2[:, jj, off:],
                        start=(j == 0),
                        stop=(j == jmax - 1),
                    )
            # single softmax-sum matmul per head
            nc.tensor.matmul(psum_l, lhsT=ones_bf_sb[:], rhs=acc,
                             start=True, stop=True)
            rec = p2rb.tile([P, SG], f32, tag="rec")
            if g == nsg - 1 and h == HPG - 1:
                for c in range(4):
                    cs = slice(c * P, (c + 1) * P)
                    nc.vector.reciprocal_approx_fast(rec[:, cs], psum_l[:, cs])
                    nc.vector.tensor_mul(ctx_sb[:, h, g * SG + c * P:
                                                g * SG + (c + 1) * P],
                                         pctx[:, cs], rec[:, cs])
            else:
                nc.vector.reciprocal_approx_fast(rec, psum_l)
                nc.vector.tensor_mul(ctx_sb[:, h, qsl], pctx, rec)

        def emit_outproj(g):
            for qt in range(4 * g, 4 * (g + 1)):
                ob = outp.tile([P, dout], bf, tag="ob")
                for dsl in range(nos):
                    po = ps_main.tile([P, SG], f32, tag="ps")
                    for h in range(HPG):
                        nc.tensor.matmul(
                            po,
                            lhsT=ctx_sb[:, h, qt * P:(qt + 1) * P],
                            rhs=wo_sb[:, h, dsl * SG:(dsl + 1) * SG],
                            start=(h == 0),
                            stop=(h == HPG - 1),
                        )
                    nc.vector.tensor_copy(ob[:, dsl * SG:(dsl + 1) * SG], po)
                    if dsl % 2 == 1:
                        c0 = (dsl - 1) * SG
                        nc.sync.dma_start(
                            io["out"][qt * P:(qt + 1) * P, c0:c0 + 2 * SG],
                            ob[:, c0:c0 + 2 * SG],
                        )

        for g in range(nsg):
            emit_head(g, 0)
            if g > 0:
                emit_outproj(g - 1)
            for h in range(1, HPG):
                emit_head(g, h)
        emit_outproj(nsg - 1)

    for pool in (ps_sum, ps2, ps_main, stores, const):
        pool.release()


def build_program(cfg=None):
    import concourse.bacc as bacc
    import concourse.mybir as mybir
    import concourse.tile as tile

    cfg = cfg or {"S": S, "D": D, "DOUT": DOUT}
    bf = mybir.dt.bfloat16
    nc = bacc.Bacc()
    io = {
        "xTg": nc.dram_tensor("xTg", [P, NSG * DI_T * SG], bf, kind="ExternalInput"),
        "wqkh": nc.dram_tensor("wqkh", [P, HPG * 2 * DI_T * P], bf,
                               kind="ExternalInput"),
        "wv": nc.dram_tensor("wv", [P, DI_T * DG], bf, kind="ExternalInput"),
        "wo": nc.dram_tensor("wo", [P, HPG * DOUT], bf, kind="ExternalInput"),
        "cosT": nc.dram_tensor("cosT", [P, cfg["S"]], bf, kind="ExternalInput"),
        "sinT": nc.dram_tensor("sinT", [P, cfg["S"]], bf, kind="ExternalInput"),
        "masks": nc.dram_tensor("masks", [P, HG, SG], bf, kind="ExternalInput"),
        "ones_bf": nc.dram_tensor("ones_bf", [P, P], bf, kind="ExternalInput"),
        "out": nc.dram_tensor(
            "out", [cfg["S"], cfg["DOUT"]], bf, kind="ExternalOutput"
        ),
    }
    with tile.TileContext(nc) as tc:
        for rep in range(cfg.get("repeat", 1)):
            _emit(tc, io, cfg, sfx=f"_r{rep}")
    nc.finalize()
    return nc


def host_constants(s=S):
    inv = 1.0 / (10000.0 ** (np.arange(0, HEAD, 2, dtype=np.float32) / HEAD))
    pos = np.arange(s, dtype=np.float32)
    ang = pos[:, None] * inv[None, :]
    ang = np.concatenate([ang, ang], axis=-1)          # (s, HEAD)
    cosT = np.cos(ang).T.astype(BF16).copy()           # (HEAD, s)
    sinT = np.sin(ang).T.astype(np.float32)
    # preshifted signed sin for DVE rotate-half:
    #   rope[d<64]  = q[d]*cos[d] - q[d+64]*sin[d]  -> sinsh[64:] = -sin[:64]
    #   rope[d>=64] = q[d]*cos[d] + q[d-64]*sin[d]  -> sinsh[:64] =  sin[64:]
    sinsh = np.empty_like(sinT)
    sinsh[0:64] = sinT[64:128]
    sinsh[64:128] = -sinT[0:64]
    sinshT = sinsh.astype(BF16).copy()
    kk = np.arange(P)[:, None, None]
    rr = np.arange(HG)[None, :, None]
    qq = np.arange(SG)[None, None, :]
    masks = (kk <= qq - P * rr).astype(BF16)           # (P, HG, SG)
    ones_bf = np.ones((P, P), BF16)
    return cosT, sinshT, masks, ones_bf


def make_in_maps(x, W_query, W_key, W_value, W_out):
    """Host-pack all inputs into DMA-friendly per-partition-contiguous
    layouts and build the 8 per-core input dicts."""
    cosT, sinshT, masks, ones_bf = host_constants()

    def pack_x(xb):
        # xTg[p, g, o, c] = x[g*SG+c, o*P+p]
        t = np.asarray(xb).reshape(NSG, SG, DI_T, P).transpose(3, 0, 2, 1)
        return np.ascontiguousarray(t).reshape(P, -1).astype(BF16)

    def pack_wqk(wq, wk, gsl):
        # wqkh[p, h, half, t, o, m] = w_t[(half*8+o)*P+p, gsl.start + h*P+m]
        hdi = DI_T // 2
        parts = []
        for w in (wq, wk):
            wg = np.asarray(w)[:, gsl]                   # [D, DG]
            t = wg.reshape(2, hdi, P, HPG, P).transpose(2, 3, 0, 1, 4)
            parts.append(t)                              # [p, h, half, o, m]
        t = np.stack(parts, axis=3)                      # [p, h, half, t, o, m]
        return np.ascontiguousarray(t).reshape(P, -1).astype(BF16)

    def pack_wv(w, gsl):
        # wv[p, o, n] = w[o*P+p, gsl.start+n]
        wg = np.asarray(w)[:, gsl]
        t = wg.reshape(DI_T, P, DG).transpose(1, 0, 2)
        return np.ascontiguousarray(t).reshape(P, -1).astype(BF16)

    def pack_wo(w, gsl):
        # wo[p, h, n] = w[gsl.start + h*P+p, n]  (row-shard of W_out)
        wg = np.asarray(w)[gsl, :]                       # [DG, DOUT]
        t = wg.reshape(HPG, P, DOUT).transpose(1, 0, 2)
        return np.ascontiguousarray(t).reshape(P, -1).astype(BF16)

    xg = [pack_x(np.asarray(x)[b]) for b in range(B)]
    in_maps = []
    for core in range(N_CORES):
        b, g = divmod(core, HG)
        gsl = slice(g * DG, (g + 1) * DG)
        in_maps.append({
            "xTg": xg[b],
            "wqkh": pack_wqk(W_query, W_key, gsl),
            "wv": pack_wv(W_value, gsl),
            "wo": pack_wo(W_out, gsl),
            "cosT": cosT, "sinT": sinshT, "masks": masks, "ones_bf": ones_bf,
        })
    return in_maps


def kernel(x, W_query, W_key, W_value, W_out):
    from concourse.bass_utils import run_bass_kernel_spmd

    x = np.asarray(x)
    in_dtype = x.dtype
    nc = build_program()
    in_maps = make_in_maps(x, W_query, W_key, W_value, W_out)
    res = run_bass_kernel_spmd(nc, in_maps, core_ids=list(range(N_CORES)))
    out = np.zeros((B, S, DOUT), np.float32)
    for core in range(N_CORES):
        b = core // HG
        out[b] += res.results[core]["out"]
    return out.astype(in_dtype, copy=False)
